# revision 1
# baseline (speedup 1.0000x reference)
"""Trainium2 Bass kernel for nn_PartialAttention (LN -> Q/K proj -> scaled QK^T -> exp(s - rowmax)).

Sharding: 8 cores = 2 batches x 4 query-blocks of 1024 tokens. Each core gets
ONLY its token block xT_blk [E=1024, SB=1024] in fp16, computes LayerNorm
stats + K^T/Q^T for the block via

    K^T = r (.) (Wg_k^T x) - sk (x) (r*mu) + ck (x) 1,  Wg_k = diag(gamma) Wk

packed as [128, 512] tiles (token-chunk halves on partition halves), then
AllGathers the K blocks across the 4 cores of its batch and computes
scores = Q^T.T @ K^T per 128-query tile in two rounds:
round 1 computes scores into PSUM and reduces the row max (DVE
tensor_tensor-max folds PSUM halves to fp16, Pool reduce_max finishes),
round 2 recomputes the scores (PE is cheap; PSUM banks can't hold scores
across the max) and exp(s - rowmax) goes straight to fp16 output.
"""

import os
from contextlib import ExitStack

import numpy as np

import concourse.bass as bass
import concourse.bacc as bacc
import concourse.mybir as mybir
import concourse.tile as tile
from concourse.bass import ts
from concourse.bass_utils import run_bass_kernel_spmd

F32 = mybir.dt.float32
FP16 = mybir.dt.float16
FT = mybir.ActivationFunctionType
AX = mybir.AxisListType
MUL = mybir.AluOpType.mult
MAX = mybir.AluOpType.max

E, S, B, D = 1024, 4096, 2, 64
P = 128
NE = E // P            # 8 e-chunks of 128
SB = 1024              # tokens per core (query block)
TS = 512               # token chunk; [P, TS] f32 = 1 PSUM bank
NCB = SB // TS         # 2
G = 4                  # AllGather group size (cores per batch)
NQT = SB // P          # 8 query tiles of 128
H4 = 4 * TS            # half-row of keys (4 banks)
EPS = 1e-5
SCALE = 1.0 / 8.0      # 1/sqrt(D)
GROUPS = [[0, 1, 2, 3], [4, 5, 6, 7]]
# K-block gather mechanism: "rdma" = direct cross-core SBUF writes (SDMA),
# "cc" = NRT AllGather collective (slow ~47us latency on this runtime).
GATHER = os.environ.get("GATHER", "cc")


def _body(tc, xT, wq, wk, gam, bet, bqv, bkv, cst, cstn, out):
    nc = tc.nc
    with ExitStack() as ctx:
        consts = ctx.enter_context(tc.tile_pool(name="consts", bufs=1))
        big = ctx.enter_context(tc.tile_pool(name="big", bufs=1))
        stats = ctx.enter_context(tc.tile_pool(name="stats", bufs=1))

        # ---------- parameter prep ----------
        wqt = consts.tile([P, NE, D], F32)
        nc.sync.dma_start(out=wqt, in_=wq.rearrange("(c p) d -> p c d", p=P))
        wkt = consts.tile([P, NE, D], F32)
        nc.sync.dma_start(out=wkt, in_=wk.rearrange("(c p) d -> p c d", p=P))
        gmt = consts.tile([P, NE], F32)
        nc.sync.dma_start(out=gmt, in_=gam)
        btt = consts.tile([P, NE], F32)
        nc.sync.dma_start(out=btt, in_=bet)
        bk_row = consts.tile([1, D], F32)
        nc.sync.dma_start(out=bk_row, in_=bkv.unsqueeze(0))
        bq_row = consts.tile([1, D], F32)
        nc.sync.dma_start(out=bq_row, in_=bqv.unsqueeze(0))
        # cst[P, 3] fp16: col 1 = ones; 2-col slices give one-hot rows
        cstt = consts.tile([P, 3], FP16)
        nc.sync.dma_start(out=cstt, in_=cst)
        ones_col = cstt[:, 1:2]
        negones = consts.tile([1, TS], FP16)
        nc.sync.dma_start(out=negones, in_=cstn)

        wgk = consts.tile([P, NE, D], FP16)
        wgq = consts.tile([P, NE, D], FP16)
        wk16 = consts.tile([P, NE, D], FP16)
        wq16 = consts.tile([P, NE, D], FP16)
        btt16 = consts.tile([P, NE], FP16)
        nc.gpsimd.tensor_copy(btt16, btt)
        for c in range(NE):
            nc.vector.tensor_scalar_mul(wgk[:, c, :], wkt[:, c, :], gmt[:, c : c + 1])
            nc.vector.tensor_scalar(
                wgq[:, c, :], wqt[:, c, :], gmt[:, c : c + 1], SCALE, op0=MUL, op1=MUL
            )
            nc.scalar.copy(wk16[:, c, :], wkt[:, c, :])
            nc.gpsimd.tensor_copy(wq16[:, c, :], wqt[:, c, :])

        # sk/sq/ck/cq rows [1, D] via PE column sums
        sk_row = consts.tile([1, D], FP16)
        sq_row = consts.tile([1, D], FP16)
        ck_row = consts.tile([1, D], FP16)
        cq_row = consts.tile([1, D], FP16)
        with tc.tile_pool(name="ppsum", bufs=1, space="PSUM") as pp:
            ps_par = pp.tile([1, 4 * D], F32)
            for g in range(4):
                for c in range(NE):
                    lhs = ones_col if g < 2 else btt16[:, c : c + 1]
                    rhs_g = (wgk[:, c, :], wgq[:, c, :], wk16[:, c, :], wq16[:, c, :])[g]
                    nc.tensor.matmul(
                        ps_par[:, g * D : (g + 1) * D],
                        lhsT=lhs,
                        rhs=rhs_g,
                        start=(c == 0),
                        stop=(c == NE - 1),
                        skip_group_check=True,
                    )
            nc.scalar.copy(sk_row, ps_par[:, 0 * D : 1 * D])
            nc.scalar.copy(sq_row, ps_par[:, 1 * D : 2 * D])
            nc.vector.tensor_add(ck_row, ps_par[:, 2 * D : 3 * D], bk_row)
            tmpc = stats.tile([1, D], F32)
            nc.vector.tensor_add(tmpc, ps_par[:, 3 * D : 4 * D], bq_row)
            nc.vector.tensor_scalar_mul(cq_row, tmpc, SCALE)

        # ---------- phase 1: own block -> LN stats, kTblk/qT ----------
        kTblk = big.tile([D, SB], FP16)
        qT = big.tile([D, SB], FP16)
        rb = big.tile([D, SB], F32)
        kT = big.tile([D, S], FP16)
        rmu_row = stats.tile([1, SB], FP16)
        r_dram = nc.dram_tensor("r_scratch", [SB], F32).ap()
        cc_in, _cc_in_free = tc.tile([D, SB], FP16, space="DRAM", name="cc_in")
        cc_out, _cc_out_free = tc.tile([G, D, SB], FP16, space="DRAM", name="cc_out")
        xT3 = xT.rearrange("(c p) t -> p c t", p=P)
        with (
            tc.tile_pool(name="xpool", bufs=2) as xpool,
            tc.tile_pool(name="sqpool", bufs=2) as sqpool,
            tc.tile_pool(name="kp", bufs=2, space="PSUM") as kp,
            tc.tile_pool(name="qp", bufs=2, space="PSUM") as qp,
            tc.tile_pool(name="sp", bufs=1, space="PSUM") as sp,
            tc.tile_pool(name="ep", bufs=2, space="PSUM") as ep,
            tc.tile_pool(name="ktmp", bufs=2) as ktmp_pool,
        ):
            s1p = sp.tile([NCB, TS], F32, name="s1p", tag="s1p")
            s2p = sp.tile([NCB, TS], F32, name="s2p", tag="s2p")
            xts = []
            pks = []
            for jj in range(NCB):
                xt = xpool.tile([P, NE, TS], FP16, name=f"xt{jj}", tag="xt")
                nc.sync.dma_start(out=xt, in_=xT3[:, :, ts(jj, TS)])
                xts.append(xt)
                xq2 = sqpool.tile([P, NE, TS], FP16, name=f"xq2{jj}", tag="xq2")
                nc.vector.tensor_mul(xq2[:, 0:3, :], xt[:, 0:3, :], xt[:, 0:3, :])
                nc.scalar.square(xq2[:, 3:6, :], xt[:, 3:6, :])
                nc.gpsimd.tensor_mul(xq2[:, 6:8, :], xt[:, 6:8, :], xt[:, 6:8, :])

                # K projection for this chunk
                pk = kp.tile([D, TS], F32, name=f"pk{jj}", tag="pk")
                for c in range(NE):
                    nc.tensor.matmul(
                        pk, lhsT=wgk[:, c, :], rhs=xt[:, c, :],
                        start=(c == 0), stop=(c == NE - 1),
                    )
                # S1/S2 column sums: one-hot lhs puts chunk jj in row jj
                for c in range(NE):
                    nc.tensor.matmul(
                        s1p, lhsT=cstt[:, 1 - jj : 3 - jj], rhs=xt[:, c, :],
                        start=(jj == 0 and c == 0),
                        stop=(jj == NCB - 1 and c == NE - 1),
                        skip_group_check=True,
                    )
                for c in range(NE):
                    nc.tensor.matmul(
                        s2p, lhsT=cstt[:, 1 - jj : 3 - jj], rhs=xq2[:, c, :],
                        start=(jj == 0 and c == 0),
                        stop=(jj == NCB - 1 and c == NE - 1),
                        skip_group_check=True,
                    )
                pks.append(pk)

            # LN stats for both chunks at once ([2, TS] tiles)
            mu = stats.tile([NCB, TS], F32)
            nc.vector.tensor_scalar_mul(mu, s1p, 1.0 / E)
            e2 = stats.tile([NCB, TS], F32)
            nc.vector.tensor_scalar_mul(e2, s2p, 1.0 / E)
            msq = stats.tile([NCB, TS], F32)
            nc.vector.tensor_mul(msq, mu, mu)
            vart = stats.tile([NCB, TS], F32)
            nc.vector.tensor_sub(vart, e2, msq)
            epsb = stats.tile([NCB, 1], F32)
            nc.vector.memset(epsb, EPS)
            sd = stats.tile([NCB, TS], F32)
            nc.scalar.activation(sd, vart, FT.Sqrt, bias=epsb[:, 0:1])
            rh = stats.tile([NCB, TS], F32)
            nc.vector.reciprocal(rh, sd)
            rmu2 = stats.tile([NCB, TS], FP16)
            nc.vector.tensor_mul(rmu2, rh, mu)
            nc.sync.dma_start(out=rmu_row, in_=rmu2)
            nc.sync.dma_start(out=r_dram, in_=rh)
            r_bc = bass.AP(
                tensor=r_dram.tensor, offset=r_dram.offset, ap=[[0, D], [1, SB]]
            )
            nc.sync.dma_start(out=rb, in_=r_bc)

            # K epilogue per chunk
            for jj in range(NCB):
                ob = ep.tile([D, TS], F32, name=f"obk{jj}", tag="ob")
                nc.tensor.matmul(ob, lhsT=sk_row, rhs=rmu_row[:, ts(jj, TS)], start=True, stop=False)
                nc.tensor.matmul(ob, lhsT=ck_row, rhs=negones, start=False, stop=True)
                tmp = ktmp_pool.tile([D, TS], F32, name=f"tmpk{jj}", tag="tmp")
                nc.vector.tensor_mul(tmp, rb[:, ts(jj, TS)], pks[jj])
                nc.vector.tensor_sub(kTblk[:, ts(jj, TS)], tmp, ob)

            # ship own K block; gather the batch's 4 blocks
            nc.gpsimd.dma_start(out=cc_in, in_=kTblk)
            nc.gpsimd.collective_compute(
                "AllGather",
                mybir.AluOpType.bypass,
                replica_groups=GROUPS,
                ins=[cc_in.opt()],
                outs=[cc_out.opt()],
            )

            # Q projection + epilogue (overlaps the collective)
            for jj in range(NCB):
                pq = qp.tile([D, TS], F32, name=f"pq{jj}", tag="pq")
                for c in range(NE):
                    nc.tensor.matmul(
                        pq, lhsT=wgq[:, c, :], rhs=xts[jj][:, c, :],
                        start=(c == 0), stop=(c == NE - 1),
                    )
                obq = ep.tile([D, TS], F32, name=f"obq{jj}", tag="ob")
                nc.tensor.matmul(obq, lhsT=sq_row, rhs=rmu_row[:, ts(jj, TS)], start=True, stop=False)
                nc.tensor.matmul(obq, lhsT=cq_row, rhs=negones, start=False, stop=True)
                tmpq = ktmp_pool.tile([D, TS], F32, name=f"tmpq{jj}", tag="tmp")
                nc.vector.tensor_mul(tmpq, rb[:, ts(jj, TS)], pq)
                nc.vector.tensor_sub(qT[:, ts(jj, TS)], tmpq, obq)

            # rank order: column block g holds keys of global block g
            for g in range(G):
                nc.sync.dma_start(out=kT[:, g * SB : (g + 1) * SB], in_=cc_out[g])

        # ---------- phase 2: scores -> e=exp(s) -> rowmax(e) -> scale ----------
        # DVE reads PSUM ~2.3c/elem but SBUF fp16 at 1c/elem, so exp first
        # (ACT reads PSUM at full speed), then max/scale on fp16 in SBUF.
        # Pool folds the max tree; the scale pass alternates DVE/ACT.
        with (
            tc.tile_pool(name="scorep", bufs=2, space="PSUM") as scorep,
            tc.tile_pool(name="outp", bufs=3) as outp,
            tc.tile_pool(name="smp", bufs=2) as smp,
            tc.tile_pool(name="mxp", bufs=2) as mxp,
        ):
            for m in range(NQT):
                lhs_q = qT[:, ts(m, P)]
                e_t = outp.tile([P, S], FP16, name=f"e{m}", tag="e")
                for h in range(2):
                    ps = scorep.tile([P, H4], F32, name=f"s{m}_{h}", tag="s")
                    for jj in range(4):
                        j = h * 4 + jj
                        nc.tensor.matmul(
                            ps[:, ts(jj, TS)],
                            lhsT=lhs_q,
                            rhs=kT[:, ts(j, TS)],
                            start=True, stop=True, skip_group_check=True,
                        )
                    nc.scalar.activation(e_t[:, h * H4 : (h + 1) * H4], ps, FT.Exp)
                # all fold operands 1024-wide: DVE TensorTensor fast mode
                # requires operand free-size <= 1024 (2048-wide measured 4.7us,
                # 1024-wide 0.8us)
                efa = smp.tile([P, 2 * TS], FP16, name=f"efa{m}", tag="efa")
                nc.vector.tensor_max(efa, e_t[:, 0 : 2 * TS], e_t[:, 2 * TS : H4])
                efb = smp.tile([P, 2 * TS], FP16, name=f"efb{m}", tag="efb")
                nc.vector.tensor_max(efb, e_t[:, H4 : H4 + 2 * TS], e_t[:, H4 + 2 * TS : 2 * H4])
                ef2 = smp.tile([P, 2 * TS], FP16, name=f"ef2{m}", tag="ef2")
                nc.vector.tensor_max(ef2, efa, efb)
                mx = mxp.tile([P, 1], F32, name=f"mx{m}", tag="mx")
                nc.vector.reduce_max(mx, ef2, axis=AX.X)
                rmx = mxp.tile([P, 1], F32, name=f"rmx{m}", tag="rmx")
                nc.vector.reciprocal(rmx, mx)
                if m % 4 == 3:
                    nc.scalar.mul(e_t, e_t, rmx[:, 0:1])
                else:
                    nc.vector.tensor_scalar_mul(e_t, e_t, rmx)
                nc.sync.dma_start(out=out[ts(m, P), :], in_=e_t)


def _build_nc():
    nc = bacc.Bacc("TRN2", target_bir_lowering=False, debug=False, num_devices=8)
    xT = nc.dram_tensor("xT", [E, SB], FP16, kind="ExternalInput").ap()
    wq = nc.dram_tensor("Wq", [E, D], F32, kind="ExternalInput").ap()
    wk = nc.dram_tensor("Wk", [E, D], F32, kind="ExternalInput").ap()
    gam = nc.dram_tensor("gamma", [P, NE], F32, kind="ExternalInput").ap()
    bet = nc.dram_tensor("beta", [P, NE], F32, kind="ExternalInput").ap()
    bqv = nc.dram_tensor("bq", [D], F32, kind="ExternalInput").ap()
    bkv = nc.dram_tensor("bk", [D], F32, kind="ExternalInput").ap()
    cst = nc.dram_tensor("cst", [P, 3], FP16, kind="ExternalInput").ap()
    cstn = nc.dram_tensor("cstn", [1, TS], FP16, kind="ExternalInput").ap()
    out = nc.dram_tensor("out", [SB, S], FP16, kind="ExternalOutput").ap()
    with tile.TileContext(nc) as tc:
        _body(tc, xT, wq, wk, gam, bet, bqv, bkv, cst, cstn, out)
    nc.compile()
    return nc


def _prepare_in_maps(src_emb, gamma, beta, Wq, bq, Wk, bk):
    src_emb = np.asarray(src_emb, np.float32)
    gamma = np.asarray(gamma, np.float32)
    beta = np.asarray(beta, np.float32)
    Wq = np.asarray(Wq, np.float32)
    bq = np.asarray(bq, np.float32)
    Wk = np.asarray(Wk, np.float32)
    bk = np.asarray(bk, np.float32)

    gamma_r = np.ascontiguousarray(gamma.reshape(NE, P).T)
    beta_r = np.ascontiguousarray(beta.reshape(NE, P).T)
    cst_np = np.zeros((P, 3), np.float16)
    cst_np[:, 1] = 1.0
    cstn_np = np.full((1, TS), -1.0, np.float16)
    xT_all = np.transpose(src_emb, (1, 2, 0)).astype(np.float16)  # [B, E, S]
    in_maps = []
    for c in range(8):
        b, qb = c // G, c % G
        blk = np.ascontiguousarray(xT_all[b][:, qb * SB : (qb + 1) * SB])
        in_maps.append(
            {
                "xT": blk,
                "Wq": Wq,
                "Wk": Wk,
                "gamma": gamma_r,
                "beta": beta_r,
                "bq": bq,
                "bk": bk,
                "cst": cst_np,
                "cstn": cstn_np,
            }
        )
    return in_maps


_nc_cache = None
_last_results = None


def kernel(src_emb, gamma, beta, Wq, bq, Wk, bk):
    global _nc_cache, _last_results
    if _nc_cache is None:
        _nc_cache = _build_nc()
    nc = _nc_cache

    in_maps = _prepare_in_maps(src_emb, gamma, beta, Wq, bq, Wk, bk)
    res = run_bass_kernel_spmd(nc, in_maps, core_ids=list(range(8)))
    _last_results = res

    full = np.empty((B, S, S), np.float32)
    for c in range(8):
        b, qb = c // G, c % G
        blk = np.asarray(res.results[c]["out"], np.float32)
        rows = slice(qb * SB, (qb + 1) * SB)
        if GATHER == "rdma":
            for i in range(G):
                g = qb ^ i
                full[b, rows, g * SB : (g + 1) * SB] = blk[:, i * SB : (i + 1) * SB]
        else:
            full[b, rows, :] = blk
    return full

